# revision 23
# baseline (speedup 1.0000x reference)
"""DEQ fixed-point kernel for Trainium2, 8-core data-parallel (fp8 streams).

Reference: 12 Broyden steps on g(z) = tanh(z W + x U + b) - z from z0 = 0, then
one final layer pass.  The map is a strong contraction (~0.27/step), so plain
Picard iteration z <- tanh(z W + c) converges; intermediate-round errors are
contracted by later rounds, so only the last rounds need accurate operands.

The kernel is DMA-bound: 16.8 MB of weight traffic (vs 21.4 MB for the bf16
baseline) as four fp8 matrices, split hi/lo:
  Uh3 = e3m4(64U), Ul3 = e3m4(32(64U - Uh3))   e3m4 = 4 mantissa bits
  Wh8 = e4m3(64W), Wl8 = e4m3(64W - Wh8)
  xa3 = e3m4(x),   xb3 = e3m4(16(x - xa3))     stationary, tiny
  c64 = xa3@Uh3 + 64b + (xb3@Uh3)/16 [cH] + (xa3@Ul3)/32 [cF]
        (3 PSUM scale groups, merged on DVE; injected per round as bf16
        hi/lo identity quartets)
  rounds: psum = c-inject + z_bf16 @ Wh8 [+ z @ Wl8 last]; z = tanh(psum/64)
z stays bf16: mixed bf16-stationary x fp8-moving matmuls run at bf16 speed,
and fp8 DoubleRow cannot target banded PSUM partitions (ISA: DR dst partition
base must be 0), so fp8 buys DMA bytes, not PE time.

Hardware findings baked into this structure (each cost ~10-15us when wrong):
  - DVE-written PSUM is NOT seen by matmul start=False accumulation on real
    HW (works in CoreSim) -> c is injected via identity quartets, no preload.
  - The HWDGE fair-shares its 16 engines across ALL in-flight descriptors,
    so a descriptor's completion semaphore fires ~in_flight_bytes/384GBps
    after issue.  Tiny tensors must therefore be issued BEFORE the big
    streams (an xat issued after 8 x 512KB chunks completes ~14us late and
    gates the first prologue quartet), and chunk-paced consumers run a few
    us behind the wire.  Both pool-gated (bufs=4) streaming and issue-rate
    pacing measured slower than plain eager dedicated-tile issue.
  - The PE sits at the ~1.2-1.3GHz p-state (~380ns per 512-col quartet)
    for chunk-paced work; zero-data filler matmuls to hold the p-state
    serialize on their column group and measured slower.

DMA order on the single sync HWDGE queue, every chunk a dedicated SBUF tile
so dma_starts issue eagerly:  smalls -> Uh3 -> Wh8 -> Ul3 -> Wl8 (fine 1-kc
chunks).  The round schedule tracks arrivals: the prologue paces Uh3; round 2
paces Wh8; the c-lo pass (ps3) follows the stale-c fast rounds and paces Ul3;
the acc round follows the cF merge; the final round's Wl8 pass paces the
stream tail.  Post-stream rounds emit all 4 transposes first, then quartets
j-major, to avoid per-block DVE bubbles.

DEQ_NCH stale-c rounds (default 2 -> 5 tanh total) and DEQ_NCF full-c rounds
(default 0) set the accuracy margin.  Measured on HW: 75.2us exec, relmax
1.15e-2 vs the fixed-seed reference (gate 2e-2; numpy-simulated relmax
matches HW to 3 digits).  bf16 Broyden-replacement baseline: 111us, 3.5e-3.
"""

import os
import sys
from contextlib import ExitStack

import numpy as np

for _p in ("/opt/trn_rl_repo",):
    try:
        import concourse  # noqa: F401
        break
    except ImportError:
        if _p not in sys.path and os.path.isdir(_p):
            sys.path.insert(0, _p)

import ml_dtypes

import concourse.bacc as bacc
import concourse.bass as bass  # noqa: F401
import concourse.tile as tile
from concourse import bass_utils, mybir

BF16 = ml_dtypes.bfloat16
E4M3 = ml_dtypes.float8_e4m3
E3M4 = ml_dtypes.float8_e3m4
F32 = mybir.dt.float32
BF = mybir.dt.bfloat16
F8 = mybir.dt.float8e4
F83 = mybir.dt.float8e3
ALU = mybir.AluOpType
ACTF = mybir.ActivationFunctionType

NCORES = 8
B, D = 256, 2048
NB = B // NCORES          # 32 batch rows per core
DC = 128 // NB            # 4 d-chunks packed along partitions
F = D // DC               # 512 free elements per partition
KC = D // 128             # 16 contraction chunks of 128
NG = D // 512             # 4 output column groups of 512

S = 64.0                  # global scale: PSUM holds 64*(zW + c)
SB = 16.0                 # x-lo scale
SU = 32.0                 # U-lo scale


def _cfg():
    return (int(os.environ.get("DEQ_NCH", "2")),
            int(os.environ.get("DEQ_NCF", "0")))


def _pack_t(a):
    """[D, NB] (d-major) -> [128, KC*NB]; stationary x layout."""
    return np.ascontiguousarray(
        a.reshape(KC, 128, NB).transpose(1, 0, 2).reshape(128, KC * NB))


def _pack_w(a):
    """[D, D] -> [128, KC*D]: row d = kc*128 + p at (p, kc, :)."""
    return np.ascontiguousarray(
        a.reshape(KC, 128, D).transpose(1, 0, 2).reshape(128, KC * D))


def _pack_state(a):
    return np.ascontiguousarray(
        a.reshape(NB, DC, F).transpose(1, 0, 2).reshape(128, F))


def _unpack_state(a):
    return np.ascontiguousarray(
        a.reshape(DC, NB, F).transpose(1, 0, 2).reshape(NB, D))


def _build(nc, zero_x0, n_ch, n_cf):
    din = {}
    shapes = [
        ("ident", [128, 128], BF), ("bstb", [128, F], BF),
        ("xat", [128, KC * NB], F83), ("xbt", [128, KC * NB], F83),
        ("uh3", [128, KC * D], F83), ("ul3", [128, KC * D], F83),
        ("wh8", [128, KC * D], F8), ("wl8", [128, KC * D], F8),
    ]
    if not zero_x0:
        shapes += [("x0ht", [128, KC * NB], BF), ("x0lt", [128, KC * NB], BF)]
    for name, shape, dt in shapes:
        din[name] = nc.dram_tensor(name, shape, dt, kind="ExternalInput").ap()
    out_dram = nc.dram_tensor("out", [128, F], F32, kind="ExternalOutput").ap()
    zslv = nc.dram_tensor("zslv", [1, 32], BF, kind="ExternalOutput").ap()

    with tile.TileContext(nc) as tc, ExitStack() as ctx:
        consts = ctx.enter_context(tc.tile_pool(name="consts", bufs=1))
        st = ctx.enter_context(tc.tile_pool(name="state", bufs=2))
        ztp = ctx.enter_context(tc.tile_pool(name="ztp", bufs=2))
        pp_c = ctx.enter_context(tc.tile_pool(name="pc", bufs=1, space="PSUM"))
        pp_z = ctx.enter_context(tc.tile_pool(name="pz", bufs=2, space="PSUM"))
        pp_t = ctx.enter_context(tc.tile_pool(name="pt", bufs=2, space="PSUM"))

        # ---- resident tiles ------------------------------------------------
        ident = consts.tile([128, 128], BF)
        bstb = consts.tile([128, F], BF)
        xat = consts.tile([128, KC, NB], F83)
        xbt = consts.tile([128, KC, NB], F83)
        uh = consts.tile([128, KC, D], F83)
        ul = consts.tile([128, KC, D], F83)
        wh = consts.tile([128, KC, D], F8)
        wl = consts.tile([128, KC, D], F8)

        # ---- DMA queue -----------------------------------------------------
        # Uh3 first (starts the stream ~2.6us earlier than smalls-first);
        # issue rate (~0.65us/chunk) is 2x the transfer rate, so later
        # descriptors queue ahead of the engines.  Wl8 last, in fine 1-kc
        # chunks so the final round's Wl pass paces the stream tail.
        def stream(name, t_, step):
            dr_ = din[name].rearrange("p (k d) -> p k d", d=D)
            for k in range(0, KC, step):
                nc.sync.dma_start(out=t_[:, k:k + step], in_=dr_[:, k:k + step])
        nc.sync.dma_start(out=ident, in_=din["ident"])
        nc.sync.dma_start(out=bstb, in_=din["bstb"])
        for nm, t_ in (("xat", xat), ("xbt", xbt)):
            nc.sync.dma_start(
                out=t_, in_=din[nm].rearrange("p (kc b) -> p kc b", b=NB))
        if not zero_x0:
            x0ht = consts.tile([128, KC, NB], BF)
            x0lt = consts.tile([128, KC, NB], BF)
            for nm, t_ in (("x0ht", x0ht), ("x0lt", x0lt)):
                nc.sync.dma_start(
                    out=t_, in_=din[nm].rearrange("p (kc b) -> p kc b", b=NB))
        stream("uh3", uh, 2)
        stream("wh8", wh, 2)
        stream("ul3", ul, 2)

        # ---- matmul helpers ------------------------------------------------
        def mk_mm(ps, total):
            cnt = [0] * NG

            def mm(lhsT, rhs, ng):
                nc.tensor.matmul(
                    ps[32 * ng:32 * (ng + 1), :], lhsT, rhs,
                    start=(cnt[ng] == 0), stop=(cnt[ng] == total - 1),
                    tile_position=(0, 32 * ng), skip_group_check=True)
                cnt[ng] += 1
            return mm

        def w_pass(mm, zt_, wt, kcs=None):
            for kc in (range(KC) if kcs is None else kcs):
                for ng in range(NG):
                    mm(zt_[:, kc, :], wt[:, kc, 512 * ng:512 * (ng + 1)], ng)

        # ---- prologue group 1 (paced by Uh3): ps1 = xa@Uh + b, ps2 = xb@Uh -
        ps1 = pp_c.tile([128, F], F32, tag="c1")
        ps2 = pp_c.tile([128, F], F32, tag="c2")
        mm1 = mk_mm(ps1, 1 + KC)
        mm2 = mk_mm(ps2, KC)
        for kc in range(KC):
            w_pass(mm1, xat, uh, [kc])
            w_pass(mm2, xbt, uh, [kc])
        for ng in range(NG):     # b last: doesn't gate the stream start
            mm1(ident[:, 32 * ng:32 * (ng + 1)], bstb, ng)

        # cH merge on DVE (one PSUM operand per op -> stage through SBUF)
        tmpa = consts.tile([128, F], F32)
        tmpb = consts.tile([128, F], F32)
        chi = consts.tile([128, F], BF)
        clo = consts.tile([128, F], BF)
        nc.vector.tensor_copy(tmpa, ps1)
        nc.vector.scalar_tensor_tensor(
            tmpb, ps2, 1.0 / SB, tmpa, op0=ALU.mult, op1=ALU.add)
        nc.vector.tensor_copy(chi, tmpb)
        nc.vector.scalar_tensor_tensor(
            clo, tmpb, 0.0, chi, op0=ALU.bypass, op1=ALU.subtract)

        # ---- round machinery ----------------------------------------------
        def open_round(ctiles, n_pass):
            ps = pp_z.tile([128, F], F32, tag="zw")
            mm = mk_mm(ps, len(ctiles) + n_pass * KC)
            for csb in ctiles:
                for ng in range(NG):
                    mm(ident[:, 32 * ng:32 * (ng + 1)], csb, ng)
            return ps, mm

        def boundary(ps_prev, wq_emit, act_in=None, paced=False):
            """tanh prev psum -> z bf16; transpose/repack into zt; emit this
            round's W quartets interleaved behind the transposes."""
            z = st.tile([128, F], BF, tag="z")
            zt = ztp.tile([128, KC, NB], BF, tag="zt")
            tp = pp_t.tile([128, NG, 128], BF, tag="tp")
            src = ps_prev if act_in is None else act_in
            nc.scalar.activation(z[:, 0:128], src[:, 0:128], ACTF.Tanh,
                                 scale=1.0 / S)
            nc.scalar.activation(z[:, 128:512], src[:, 128:512], ACTF.Tanh,
                                 scale=1.0 / S)

            def tr(j):
                nc.tensor.transpose(
                    tp[:, j, :], z[:, 128 * j:128 * (j + 1)], ident)
                nc.vector.tensor_copy(zt[:, j::NG, :], tp[:, j, :])

            if paced:
                tr(0)
                wq_emit(zt, [0])
                tr(1)
                wq_emit(zt, [1])
                tr(2)
                wq_emit(zt, [2])
                tr(3)
                wq_emit(zt, list(range(3, KC)))
            else:
                # all transposes first; DVE repacks pipeline behind them, so
                # the j-major quartet stream then runs without per-block
                # bubbles (keeps the PE ramped)
                tr(0), tr(1), tr(2), tr(3)
                for j in range(NG):
                    wq_emit(zt, list(range(j, KC, NG)))
            return zt, z

        def fast_round(ps_prev, ctiles, act_in=None, paced=False):
            ps, mm = open_round(ctiles, 1)

            def emit(zt, kcs, mm=mm):
                w_pass(mm, zt, wh, kcs)
            _, z = boundary(ps_prev, emit, act_in=act_in, paced=paced)
            return ps, z

        # ---- round 1 + stale-c fast rounds (overlap Wh8 / Ul3 streams) ----
        if zero_x0:
            ps_prev, z1 = fast_round(None, (chi, clo), act_in=tmpb, paced=True)
        else:
            ps0, mm0 = open_round((chi, clo), 2)
            w_pass(mm0, x0ht, wh)
            w_pass(mm0, x0lt, wh)
            ps_prev, z1 = fast_round(ps0, (chi, clo), paced=True)
        # Gate the Wl8 issue behind round 1: a 64B sliver store makes the
        # in-order sync queue hold Wl8's 16 descriptors until ~r1, so they
        # don't steal HWDGE fair-share from the Wh8/Ul3 streams (the final
        # round doesn't consume Wl8 until long after).
        nc.sync.dma_start(out=zslv, in_=z1[0:1, 0:32])
        stream("wl8", wl, 1)
        for r in range(n_ch - 1):
            ps_prev, _ = fast_round(ps_prev, (chi, clo))

        # ---- c-lo group (paced by Ul3): ps3 = xa@Ul ------------------------
        ps3 = pp_c.tile([128, F], F32, tag="c3")
        mm3 = mk_mm(ps3, KC)
        w_pass(mm3, xat, ul)

        # cF merge
        tmpd = consts.tile([128, F], F32)
        chF = consts.tile([128, F], BF)
        clF = consts.tile([128, F], BF)
        nc.vector.scalar_tensor_tensor(
            tmpd, ps3, 1.0 / SU, tmpb, op0=ALU.mult, op1=ALU.add)
        nc.vector.tensor_copy(chF, tmpd)
        nc.vector.scalar_tensor_tensor(
            clF, tmpd, 0.0, chF, op0=ALU.bypass, op1=ALU.subtract)

        # ---- full-c rounds + acc round ------------------------------------
        for r in range(n_cf + 1):
            ps_prev, _ = fast_round(ps_prev, (chF, clF))

        # ---- final round: z @ (Wh + Wl), Wl chunk-paced against the tail --
        ps, mm = open_round((chF, clF), 2)
        fin_zt = []

        def emit_fin(zt, kcs, mm=mm):
            w_pass(mm, zt, wh, kcs)
            fin_zt.append(zt)
        boundary(ps_prev, emit_fin)
        w_pass(mm, fin_zt[0], wl)
        ps_prev = ps

        # ---- output -------------------------------------------------------
        zo = st.tile([128, F], F32, tag="zo")
        for h in range(2):
            blk = slice(256 * h, 256 * (h + 1))
            nc.scalar.activation(zo[:, blk], ps_prev[:, blk], ACTF.Tanh,
                                 scale=1.0 / S)
            nc.sync.dma_start(out=out_dram[:, blk], in_=zo[:, blk])

    return nc


_CACHE = {}


def _get_nc(zero_x0=True):
    n_ch, n_cf = _cfg()
    key = ("nc", bool(zero_x0), n_ch, n_cf)
    if key not in _CACHE:
        nc = bacc.Bacc("TRN2", target_bir_lowering=False, debug=False,
                       enable_asserts=False, num_devices=NCORES)
        _build(nc, zero_x0, n_ch, n_cf)
        nc.compile()
        _CACHE[key] = nc
    return _CACHE[key]


def make_in_maps(x, initial_point, W, U, b, zero_x0):
    x = np.asarray(x, np.float32)
    x0 = np.asarray(initial_point, np.float32)
    W = np.asarray(W, np.float32)
    U = np.asarray(U, np.float32)
    b = np.asarray(b, np.float32)

    uh3 = (S * U).astype(E3M4)
    ul3 = (SU * (S * U - uh3.astype(np.float32))).astype(E3M4)
    wh8 = (S * W).astype(E4M3)
    wl8 = (S * W - wh8.astype(np.float32)).astype(E4M3)
    bstb = np.repeat((S * b).reshape(DC, 1, F), NB, axis=1).reshape(128, F)
    shared = dict(
        uh3=_pack_w(uh3), ul3=_pack_w(ul3),
        wh8=_pack_w(wh8), wl8=_pack_w(wl8),
        bstb=bstb.astype(BF16), ident=np.eye(128, dtype=BF16),
    )
    in_maps = []
    for i in range(NCORES):
        rows = slice(i * NB, (i + 1) * NB)
        xa = x[rows].astype(E3M4)
        xb = (SB * (x[rows] - xa.astype(np.float32))).astype(E3M4)
        m = dict(shared, xat=_pack_t(xa.T), xbt=_pack_t(xb.T))
        if not zero_x0:
            x0h = x0[rows].astype(BF16)
            x0l = (x0[rows] - x0h.astype(np.float32)).astype(BF16)
            m["x0ht"] = _pack_t(x0h.T)
            m["x0lt"] = _pack_t(x0l.T)
        in_maps.append(m)
    return in_maps


def run_full(inputs, trace=False):
    zero_x0 = not np.any(np.asarray(inputs["initial_point"]))
    nc = _get_nc(zero_x0)
    in_maps = make_in_maps(**inputs, zero_x0=zero_x0)
    res = bass_utils.run_bass_kernel_spmd(
        nc, in_maps, core_ids=list(range(NCORES)), trace=trace)
    out = np.concatenate(
        [_unpack_state(np.asarray(r["out"], np.float32).reshape(128, F))
         for r in res.results], axis=0)
    return out, res


def kernel(x, initial_point, W, U, b):
    out, _ = run_full(dict(x=x, initial_point=initial_point, W=W, U=U, b=b))
    return out


# revision 24
# speedup vs baseline: 1.0418x; 1.0418x over previous
"""DEQ fixed-point kernel for Trainium2, 8-core data-parallel (fp8 streams).

Reference: 12 Broyden steps on g(z) = tanh(z W + x U + b) - z from z0 = 0, then
one final layer pass.  The map is a strong contraction (~0.27/step), so plain
Picard iteration z <- tanh(z W + c) converges; intermediate-round errors are
contracted by later rounds, so only the last rounds need accurate operands.

The kernel is DMA-bound: 16.8 MB of weight traffic (vs 21.4 MB for the bf16
baseline) as four fp8 matrices, split hi/lo:
  Uh3 = e3m4(64U), Ul3 = e3m4(32(64U - Uh3))   e3m4 = 4 mantissa bits
  Wh8 = e4m3(64W), Wl8 = e4m3(64W - Wh8)
  xa3 = e3m4(x),   xb3 = e3m4(16(x - xa3))     stationary, tiny
  c64 = xa3@Uh3 + 64b + (xb3@Uh3)/16 [cH] + (xa3@Ul3)/32 [cF]
        (3 PSUM scale groups, merged on DVE; injected per round as bf16
        hi/lo identity quartets)
  rounds: psum = c-inject + z_bf16 @ Wh8 [+ z @ Wl8 last]; z = tanh(psum/64)
z stays bf16: mixed bf16-stationary x fp8-moving matmuls run at bf16 speed,
and fp8 DoubleRow cannot target banded PSUM partitions (ISA: DR dst partition
base must be 0), so fp8 buys DMA bytes, not PE time.

Hardware findings baked into this structure (each cost ~10-15us when wrong):
  - DVE-written PSUM is NOT seen by matmul start=False accumulation on real
    HW (works in CoreSim) -> c is injected via identity quartets, no preload.
  - The HWDGE fair-shares its 16 engines across ALL in-flight descriptors,
    so a descriptor's completion semaphore fires ~in_flight_bytes/384GBps
    after issue.  Tiny tensors must therefore be issued BEFORE the big
    streams (an xat issued after 8 x 512KB chunks completes ~14us late and
    gates the first prologue quartet), and chunk-paced consumers run a few
    us behind the wire.  Both pool-gated (bufs=4) streaming and issue-rate
    pacing measured slower than plain eager dedicated-tile issue.
  - The PE sits at the ~1.2-1.3GHz p-state (~380ns per 512-col quartet)
    for chunk-paced work; zero-data filler matmuls to hold the p-state
    serialize on their column group and measured slower.

DMA order on the single sync HWDGE queue, every chunk a dedicated SBUF tile
so dma_starts issue eagerly:  smalls -> Uh3 -> Wh8 -> Ul3 -> Wl8 (fine 1-kc
chunks).  The round schedule tracks arrivals: the prologue paces Uh3; round 2
paces Wh8; the c-lo pass (ps3) follows the stale-c fast rounds and paces Ul3;
the acc round follows the cF merge; the final round's Wl8 pass paces the
stream tail.  Post-stream rounds emit all 4 transposes first, then quartets
j-major, to avoid per-block DVE bubbles.

DEQ_NCH stale-c rounds (default 2 -> 5 tanh total) and DEQ_NCF full-c rounds
(default 0) set the accuracy margin.  Measured on HW: 75.2us exec, relmax
1.15e-2 vs the fixed-seed reference (gate 2e-2; numpy-simulated relmax
matches HW to 3 digits).  bf16 Broyden-replacement baseline: 111us, 3.5e-3.
"""

import os
import sys
from contextlib import ExitStack

import numpy as np

for _p in ("/opt/trn_rl_repo",):
    try:
        import concourse  # noqa: F401
        break
    except ImportError:
        if _p not in sys.path and os.path.isdir(_p):
            sys.path.insert(0, _p)

import ml_dtypes

import concourse.bacc as bacc
import concourse.bass as bass  # noqa: F401
import concourse.tile as tile
from concourse import bass_utils, mybir

BF16 = ml_dtypes.bfloat16
E4M3 = ml_dtypes.float8_e4m3
E3M4 = ml_dtypes.float8_e3m4
F32 = mybir.dt.float32
BF = mybir.dt.bfloat16
F8 = mybir.dt.float8e4
F83 = mybir.dt.float8e3
ALU = mybir.AluOpType
ACTF = mybir.ActivationFunctionType

NCORES = 8
B, D = 256, 2048
NB = B // NCORES          # 32 batch rows per core
DC = 128 // NB            # 4 d-chunks packed along partitions
F = D // DC               # 512 free elements per partition
KC = D // 128             # 16 contraction chunks of 128
NG = D // 512             # 4 output column groups of 512

S = 64.0                  # global scale: PSUM holds 64*(zW + c)
SB = 16.0                 # x-lo scale
SU = 32.0                 # U-lo scale


def _cfg():
    return (int(os.environ.get("DEQ_NCH", "2")),
            int(os.environ.get("DEQ_NCF", "0")))


def _pack_t(a):
    """[D, NB] (d-major) -> [128, KC*NB]; stationary x layout."""
    return np.ascontiguousarray(
        a.reshape(KC, 128, NB).transpose(1, 0, 2).reshape(128, KC * NB))


def _pack_w(a):
    """[D, D] -> [128, KC*D]: row d = kc*128 + p at (p, kc, :)."""
    return np.ascontiguousarray(
        a.reshape(KC, 128, D).transpose(1, 0, 2).reshape(128, KC * D))


def _pack_state(a):
    return np.ascontiguousarray(
        a.reshape(NB, DC, F).transpose(1, 0, 2).reshape(128, F))


def _unpack_state(a):
    return np.ascontiguousarray(
        a.reshape(DC, NB, F).transpose(1, 0, 2).reshape(NB, D))


def _build(nc, zero_x0, n_ch, n_cf):
    din = {}
    shapes = [
        ("ident", [128, 128], BF), ("bstb", [128, F], BF),
        ("xat", [128, KC * NB], F83), ("xbt", [128, KC * NB], F83),
        ("uh3", [128, KC * D], F83), ("ul3", [128, KC * D], F83),
        ("wh8", [128, KC * D], F8), ("wl8", [128, KC * D], F8),
    ]
    if not zero_x0:
        shapes += [("x0ht", [128, KC * NB], BF), ("x0lt", [128, KC * NB], BF)]
    for name, shape, dt in shapes:
        din[name] = nc.dram_tensor(name, shape, dt, kind="ExternalInput").ap()
    out_dram = nc.dram_tensor("out", [128, F], F32, kind="ExternalOutput").ap()

    with tile.TileContext(nc) as tc, ExitStack() as ctx:
        consts = ctx.enter_context(tc.tile_pool(name="consts", bufs=1))
        st = ctx.enter_context(tc.tile_pool(name="state", bufs=2))
        ztp = ctx.enter_context(tc.tile_pool(name="ztp", bufs=2))
        pp_c = ctx.enter_context(tc.tile_pool(name="pc", bufs=1, space="PSUM"))
        pp_z = ctx.enter_context(tc.tile_pool(name="pz", bufs=2, space="PSUM"))
        pp_t = ctx.enter_context(tc.tile_pool(name="pt", bufs=2, space="PSUM"))

        # ---- resident tiles ------------------------------------------------
        ident = consts.tile([128, 128], BF)
        bstb = consts.tile([128, F], BF)
        xat = consts.tile([128, KC, NB], F83)
        xbt = consts.tile([128, KC, NB], F83)
        uh = consts.tile([128, KC, D], F83)
        ul = consts.tile([128, KC, D], F83)
        wh = consts.tile([128, KC, D], F8)
        wl = consts.tile([128, KC, D], F8)

        # ---- DMA queue -----------------------------------------------------
        # Uh3 first (starts the stream ~2.6us earlier than smalls-first);
        # issue rate (~0.65us/chunk) is 2x the transfer rate, so later
        # descriptors queue ahead of the engines.  Wl8 last, in fine 1-kc
        # chunks so the final round's Wl pass paces the stream tail.
        def stream(name, t_, step):
            dr_ = din[name].rearrange("p (k d) -> p k d", d=D)
            for k in range(0, KC, step):
                nc.sync.dma_start(out=t_[:, k:k + step], in_=dr_[:, k:k + step])
        nc.sync.dma_start(out=ident, in_=din["ident"])
        nc.sync.dma_start(out=bstb, in_=din["bstb"])
        for nm, t_ in (("xat", xat), ("xbt", xbt)):
            nc.sync.dma_start(
                out=t_, in_=din[nm].rearrange("p (kc b) -> p kc b", b=NB))
        if not zero_x0:
            x0ht = consts.tile([128, KC, NB], BF)
            x0lt = consts.tile([128, KC, NB], BF)
            for nm, t_ in (("x0ht", x0ht), ("x0lt", x0lt)):
                nc.sync.dma_start(
                    out=t_, in_=din[nm].rearrange("p (kc b) -> p kc b", b=NB))
        stream("uh3", uh, 2)
        stream("wh8", wh, 2)
        stream("ul3", ul, 2)
        stream("wl8", wl, 1)

        # ---- matmul helpers ------------------------------------------------
        def mk_mm(ps, total):
            cnt = [0] * NG

            def mm(lhsT, rhs, ng):
                nc.tensor.matmul(
                    ps[32 * ng:32 * (ng + 1), :], lhsT, rhs,
                    start=(cnt[ng] == 0), stop=(cnt[ng] == total - 1),
                    tile_position=(0, 32 * ng), skip_group_check=True)
                cnt[ng] += 1
            return mm

        def w_pass(mm, zt_, wt, kcs=None):
            for kc in (range(KC) if kcs is None else kcs):
                for ng in range(NG):
                    mm(zt_[:, kc, :], wt[:, kc, 512 * ng:512 * (ng + 1)], ng)

        # ---- prologue group 1 (paced by Uh3): ps1 = xa@Uh + b, ps2 = xb@Uh -
        ps1 = pp_c.tile([128, F], F32, tag="c1")
        ps2 = pp_c.tile([128, F], F32, tag="c2")
        mm1 = mk_mm(ps1, 1 + KC)
        mm2 = mk_mm(ps2, KC)
        for kc in range(KC):
            w_pass(mm1, xat, uh, [kc])
            w_pass(mm2, xbt, uh, [kc])
        for ng in range(NG):     # b last: doesn't gate the stream start
            mm1(ident[:, 32 * ng:32 * (ng + 1)], bstb, ng)

        # cH merge on DVE (one PSUM operand per op -> stage through SBUF)
        tmpa = consts.tile([128, F], F32)
        tmpb = consts.tile([128, F], F32)
        chi = consts.tile([128, F], BF)
        clo = consts.tile([128, F], BF)
        nc.vector.tensor_copy(tmpa, ps1)
        nc.vector.scalar_tensor_tensor(
            tmpb, ps2, 1.0 / SB, tmpa, op0=ALU.mult, op1=ALU.add)
        nc.vector.tensor_copy(chi, tmpb)
        nc.vector.scalar_tensor_tensor(
            clo, tmpb, 0.0, chi, op0=ALU.bypass, op1=ALU.subtract)

        # ---- round machinery ----------------------------------------------
        def open_round(ctiles, n_pass):
            ps = pp_z.tile([128, F], F32, tag="zw")
            mm = mk_mm(ps, len(ctiles) + n_pass * KC)
            for csb in ctiles:
                for ng in range(NG):
                    mm(ident[:, 32 * ng:32 * (ng + 1)], csb, ng)
            return ps, mm

        def boundary(ps_prev, wq_emit, act_in=None, paced=False):
            """tanh prev psum -> z bf16; transpose/repack into zt; emit this
            round's W quartets interleaved behind the transposes."""
            z = st.tile([128, F], BF, tag="z")
            zt = ztp.tile([128, KC, NB], BF, tag="zt")
            tp = pp_t.tile([128, NG, 128], BF, tag="tp")
            src = ps_prev if act_in is None else act_in
            nc.scalar.activation(z[:, 0:128], src[:, 0:128], ACTF.Tanh,
                                 scale=1.0 / S)
            nc.scalar.activation(z[:, 128:512], src[:, 128:512], ACTF.Tanh,
                                 scale=1.0 / S)

            def tr(j):
                nc.tensor.transpose(
                    tp[:, j, :], z[:, 128 * j:128 * (j + 1)], ident)
                nc.vector.tensor_copy(zt[:, j::NG, :], tp[:, j, :])

            if paced:
                tr(0)
                wq_emit(zt, [0])
                tr(1)
                wq_emit(zt, [1])
                tr(2)
                wq_emit(zt, [2])
                tr(3)
                wq_emit(zt, list(range(3, KC)))
            else:
                # all transposes first; DVE repacks pipeline behind them, so
                # the j-major quartet stream then runs without per-block
                # bubbles (keeps the PE ramped)
                tr(0), tr(1), tr(2), tr(3)
                for j in range(NG):
                    wq_emit(zt, list(range(j, KC, NG)))
            return zt, z

        def fast_round(ps_prev, ctiles, act_in=None, paced=False):
            ps, mm = open_round(ctiles, 1)

            def emit(zt, kcs, mm=mm):
                w_pass(mm, zt, wh, kcs)
            _, z = boundary(ps_prev, emit, act_in=act_in, paced=paced)
            return ps, z

        # ---- round 1 + stale-c fast rounds (overlap Wh8 / Ul3 streams) ----
        if zero_x0:
            ps_prev, z1 = fast_round(None, (chi, clo), act_in=tmpb, paced=True)
        else:
            ps0, mm0 = open_round((chi, clo), 2)
            w_pass(mm0, x0ht, wh)
            w_pass(mm0, x0lt, wh)
            ps_prev, z1 = fast_round(ps0, (chi, clo), paced=True)
        for r in range(n_ch - 1):
            ps_prev, _ = fast_round(ps_prev, (chi, clo))

        # ---- c-lo group (paced by Ul3): ps3 = xa@Ul ------------------------
        ps3 = pp_c.tile([128, F], F32, tag="c3")
        mm3 = mk_mm(ps3, KC)
        w_pass(mm3, xat, ul)

        # cF merge
        tmpd = consts.tile([128, F], F32)
        chF = consts.tile([128, F], BF)
        clF = consts.tile([128, F], BF)
        nc.vector.scalar_tensor_tensor(
            tmpd, ps3, 1.0 / SU, tmpb, op0=ALU.mult, op1=ALU.add)
        nc.vector.tensor_copy(chF, tmpd)
        nc.vector.scalar_tensor_tensor(
            clF, tmpd, 0.0, chF, op0=ALU.bypass, op1=ALU.subtract)

        # ---- full-c rounds + acc round ------------------------------------
        for r in range(n_cf + 1):
            ps_prev, _ = fast_round(ps_prev, (chF, clF))

        # ---- final round: z @ (Wh + Wl), Wl chunk-paced against the tail --
        ps, mm = open_round((chF, clF), 2)
        fin_zt = []

        def emit_fin(zt, kcs, mm=mm):
            w_pass(mm, zt, wh, kcs)
            fin_zt.append(zt)
        boundary(ps_prev, emit_fin)
        w_pass(mm, fin_zt[0], wl)
        ps_prev = ps

        # ---- output -------------------------------------------------------
        zo = st.tile([128, F], F32, tag="zo")
        for h in range(2):
            blk = slice(256 * h, 256 * (h + 1))
            nc.scalar.activation(zo[:, blk], ps_prev[:, blk], ACTF.Tanh,
                                 scale=1.0 / S)
            nc.sync.dma_start(out=out_dram[:, blk], in_=zo[:, blk])

    return nc


_CACHE = {}


def _get_nc(zero_x0=True):
    n_ch, n_cf = _cfg()
    key = ("nc", bool(zero_x0), n_ch, n_cf)
    if key not in _CACHE:
        nc = bacc.Bacc("TRN2", target_bir_lowering=False, debug=False,
                       enable_asserts=False, num_devices=NCORES)
        _build(nc, zero_x0, n_ch, n_cf)
        nc.compile()
        _CACHE[key] = nc
    return _CACHE[key]


def make_in_maps(x, initial_point, W, U, b, zero_x0):
    x = np.asarray(x, np.float32)
    x0 = np.asarray(initial_point, np.float32)
    W = np.asarray(W, np.float32)
    U = np.asarray(U, np.float32)
    b = np.asarray(b, np.float32)

    uh3 = (S * U).astype(E3M4)
    ul3 = (SU * (S * U - uh3.astype(np.float32))).astype(E3M4)
    wh8 = (S * W).astype(E4M3)
    wl8 = (S * W - wh8.astype(np.float32)).astype(E4M3)
    bstb = np.repeat((S * b).reshape(DC, 1, F), NB, axis=1).reshape(128, F)
    shared = dict(
        uh3=_pack_w(uh3), ul3=_pack_w(ul3),
        wh8=_pack_w(wh8), wl8=_pack_w(wl8),
        bstb=bstb.astype(BF16), ident=np.eye(128, dtype=BF16),
    )
    in_maps = []
    for i in range(NCORES):
        rows = slice(i * NB, (i + 1) * NB)
        xa = x[rows].astype(E3M4)
        xb = (SB * (x[rows] - xa.astype(np.float32))).astype(E3M4)
        m = dict(shared, xat=_pack_t(xa.T), xbt=_pack_t(xb.T))
        if not zero_x0:
            x0h = x0[rows].astype(BF16)
            x0l = (x0[rows] - x0h.astype(np.float32)).astype(BF16)
            m["x0ht"] = _pack_t(x0h.T)
            m["x0lt"] = _pack_t(x0l.T)
        in_maps.append(m)
    return in_maps


def run_full(inputs, trace=False):
    zero_x0 = not np.any(np.asarray(inputs["initial_point"]))
    nc = _get_nc(zero_x0)
    in_maps = make_in_maps(**inputs, zero_x0=zero_x0)
    res = bass_utils.run_bass_kernel_spmd(
        nc, in_maps, core_ids=list(range(NCORES)), trace=trace)
    out = np.concatenate(
        [_unpack_state(np.asarray(r["out"], np.float32).reshape(128, F))
         for r in res.results], axis=0)
    return out, res


def kernel(x, initial_point, W, U, b):
    out, _ = run_full(dict(x=x, initial_point=initial_point, W=W, U=U, b=b))
    return out
